# revision 4
# baseline (speedup 1.0000x reference)
"""Trainium2 Bass kernel: log-cosine loss.

loss = mean_i[ -log((cos_sim(x_i, y_i) + 1) / 2) ]  for x, y of shape [8192, 1024].

The reference materializes the full 8192x8192 cosine matrix and takes the
diagonal; only the row-wise dots are actually needed:
    xy_i = sum_d x[i,d]*y[i,d],  xx_i = sum_d x[i,d]^2,  yy_i = sum_d y[i,d]^2
    cos_i = xy_i / sqrt(xx_i * yy_i)
    loss  = mean(-log((cos_i + 1)/2))

Sharding: data-parallel over rows, 1024 rows per core across 8 cores. Each
core reduces its 1024 rows to a single partial scalar (already scaled by
-1/8192); the host sums the 8 partials.

Per-core engine split (8 tiles of [128, 1024] per input):
  - xy  -> DVE tensor_tensor_reduce (fused mult + add-reduce), 8 ops
  - xx  -> ACT activation(Square, accum_out), 8 ops
  - yy  -> first YA tiles on ACT, rest on DVE (load balance)
Epilogue avoids the sqrt table set: 1/sqrt(p) = exp(-0.5*ln(p)), keeping all
ACT work in the natural_log/exp table set.
"""

import numpy as np

N, D = 8192, 1024
NCORES = 8
R = N // NCORES  # rows per core
P = 128          # SBUF partitions
T = R // P       # row tiles per core
YA = 3           # yy tiles 0..YA-1 computed on ACT, the rest on DVE

_nc_cache = None
last_results = None  # test harness reads exec_time off this


def _build():
    from contextlib import ExitStack

    import concourse.bacc as bacc
    import concourse.tile as tile
    from concourse import mybir

    AF = mybir.ActivationFunctionType
    ALU = mybir.AluOpType
    f32 = mybir.dt.float32

    # Bacc (not raw Bass): its compile() runs generate_event_semaphores,
    # which legalizes multi-sem waits down to the TRN2 1-wait-per-inst limit.
    nc = bacc.Bacc("TRN2", target_bir_lowering=False, debug=False)
    x = nc.declare_dram_parameter("x", [R, D], f32, isOutput=False)
    y = nc.declare_dram_parameter("y", [R, D], f32, isOutput=False)
    out = nc.declare_dram_parameter("out", [1, 1], f32, isOutput=True)
    xt = x.rearrange("(t p) d -> t p d", p=P)
    yt = y.rearrange("(t p) d -> t p d", p=P)

    with ExitStack() as ctx:
        tc = ctx.enter_context(tile.TileContext(nc))
        xpool = ctx.enter_context(tc.tile_pool(name="xp", bufs=T))
        ypool = ctx.enter_context(tc.tile_pool(name="yp", bufs=T))
        scr = ctx.enter_context(tc.tile_pool(name="scr", bufs=1))
        stats = ctx.enter_context(tc.tile_pool(name="stats", bufs=1))
        psum = ctx.enter_context(tc.tile_pool(name="ps", bufs=1, space="PSUM"))

        # Elementwise-product dump targets; one per engine so the engines
        # never share a WAW dependency on them.
        scr_dve = scr.tile([P, D], f32, tag="scr_dve")
        scr_act = scr.tile([P, D], f32, tag="scr_act")

        # Per-row reduction columns, one column per row-tile. Separate
        # tiles per writing engine (xx: ACT, xy: DVE, yy split).
        xx = stats.tile([P, T], f32, tag="xx")
        xy = stats.tile([P, T], f32, tag="xy")
        yy_a = stats.tile([P, YA], f32, tag="yya")
        yy_d = stats.tile([P, T - YA], f32, tag="yyd")

        xtiles, ytiles = [], []
        for t in range(T):
            xtile = xpool.tile([P, D], f32)
            nc.sync.dma_start(out=xtile, in_=xt[t])
            ytile = ypool.tile([P, D], f32)
            nc.sync.dma_start(out=ytile, in_=yt[t])
            xtiles.append(xtile)
            ytiles.append(ytile)

        for t in range(T):
            a, b = xtiles[t], ytiles[t]
            # scalar_tensor_tensor: out = (in0 op0 s) op1 in1, accum = sum(out)
            nc.vector.scalar_tensor_tensor(
                scr_dve, a, 1.0, b, op0=ALU.mult, op1=ALU.mult,
                accum_out=xy[:, t : t + 1],
            )
            nc.scalar.activation(scr_act, a, AF.Square, accum_out=xx[:, t : t + 1])
            if t < YA:
                nc.scalar.activation(
                    scr_act, b, AF.Square, accum_out=yy_a[:, t : t + 1]
                )
            else:
                nc.vector.scalar_tensor_tensor(
                    scr_dve, b, 1.0, b, op0=ALU.mult, op1=ALU.mult,
                    accum_out=yy_d[:, t - YA : t - YA + 1],
                )

        # cos = xy / sqrt(xx*yy); loss_row = ln(0.5*cos + 0.5); out = -mean.
        prod = stats.tile([P, T], f32, tag="prod")
        nc.vector.tensor_mul(prod[:, :YA], xx[:, :YA], yy_a)
        nc.vector.tensor_mul(prod[:, YA:], xx[:, YA:], yy_d)
        # Guard against a zero row (reference clamps norms at 1e-12).
        nc.vector.tensor_scalar_max(prod, prod, 1e-32)
        lp = stats.tile([P, T], f32, tag="lp")
        nc.scalar.activation(lp, prod, AF.Ln)
        rinv = stats.tile([P, T], f32, tag="rinv")
        nc.scalar.activation(rinv, lp, AF.Exp, scale=-0.5)  # prod**-0.5
        cos = stats.tile([P, T], f32, tag="cos")
        nc.vector.tensor_mul(cos, xy, rinv)
        half = stats.tile([P, 1], f32, tag="half")
        nc.vector.memset(half, 0.5)
        terms = stats.tile([P, T], f32, tag="terms")
        row_loss = stats.tile([P, 1], f32, tag="rl")
        nc.scalar.activation(
            terms, cos, AF.Ln, scale=0.5, bias=half, accum_out=row_loss
        )
        # Partition-axis sum via PE: [1,1] = row_loss.T @ (-1/N ones).
        wvec = stats.tile([P, 1], f32, tag="wv")
        nc.vector.memset(wvec, -1.0 / N)
        acc = psum.tile([1, 1], f32, tag="acc")
        nc.tensor.matmul(acc, row_loss, wvec, start=True, stop=True)
        res = stats.tile([1, 1], f32, tag="res")
        nc.scalar.copy(res, acc)
        nc.sync.dma_start(out=out[:, :], in_=res)

    nc.compile()
    return nc


def kernel(x, y, trace=False):
    from concourse.bass_utils import run_bass_kernel_spmd

    global _nc_cache, last_results
    if _nc_cache is None:
        _nc_cache = _build()
    nc = _nc_cache

    x = np.ascontiguousarray(np.asarray(x), dtype=np.float32)
    y = np.ascontiguousarray(np.asarray(y), dtype=np.float32)
    in_maps = [
        {"x": x[c * R : (c + 1) * R], "y": y[c * R : (c + 1) * R]}
        for c in range(NCORES)
    ]
    results = run_bass_kernel_spmd(
        nc, in_maps, core_ids=list(range(NCORES)), trace=trace
    )
    last_results = results
    total = np.float32(0.0)
    for r in results.results:
        total = np.float32(total + r["out"][0, 0])
    return np.asarray(total, dtype=np.float32)


# revision 11
# speedup vs baseline: 1.0613x; 1.0613x over previous
"""Trainium2 Bass kernel: log-cosine loss.

loss = mean_i[ -log((cos_sim(x_i, y_i) + 1) / 2) ]  for x, y of shape [8192, 1024].

The reference materializes the full 8192x8192 cosine matrix and takes the
diagonal; only the row-wise dots are actually needed:
    xy_i = sum_d x[i,d]*y[i,d],  xx_i = sum_d x[i,d]^2,  yy_i = sum_d y[i,d]^2
    cos_i = xy_i / sqrt(xx_i * yy_i)
    loss  = mean(-log((cos_i + 1)/2))

Sharding: data-parallel over rows, 1024 rows per core across 8 cores. Each
core reduces its 1024 rows to a single partial scalar (already scaled by
-1/8192); the host sums the 8 partials.

Per-core engine split (8 tiles of [128, 1024] per input):
  - xy  -> DVE tensor_tensor_reduce (fused mult + add-reduce), 8 ops
  - xx  -> ACT activation(Square, accum_out), 8 ops
  - yy  -> first YA tiles on ACT, rest on DVE (load balance)
Epilogue avoids the sqrt table set: 1/sqrt(p) = exp(-0.5*ln(p)), keeping all
ACT work in the natural_log/exp table set.
"""

import os

import numpy as np

N, D = 8192, 1024
NCORES = 8
R = N // NCORES  # rows per core
P = 128          # SBUF partitions
T = R // P       # row tiles per core
YA = 3           # yy tiles 0..YA-1 computed on ACT, the rest on DVE

_nc_cache = None
last_results = None  # test harness reads exec_time off this


def _install_act_table_override():
    """Point walrus at an act_info.json containing only the
    natural_log_exp_and_others table set (ln + exp + square + copy — every
    activation this kernel uses). Walrus picks table sets greedily per
    function, which otherwise costs three ~1.3us ACT_TABLE_LOAD swaps on the
    critical path; with a single set there is exactly one load."""
    import json
    import tempfile

    if os.environ.get("BASS_ACT_ROOT_JSON_PATH"):
        return
    try:
        from neuronxcc.driver.Job import Job
        from neuronxcc.driver.jobs.support.FindActInfo import findActInfoFile

        src = findActInfoFile(Job.getPackageDir(), "gen3")
        src_dir = os.path.dirname(src)
        with open(src) as f:
            d = json.load(f)
        keep = [
            s for s in d["act_func_sets"]
            if s["name"] == "natural_log_exp_and_others"
        ]
        if not keep:
            return
        d["act_func_sets"] = keep
        dst_dir = tempfile.mkdtemp(prefix="act_override_")
        for s in keep:
            for key in d["pwp_file_keys"]:
                fn = s[key]
                os.symlink(os.path.join(src_dir, fn), os.path.join(dst_dir, fn))
        with open(os.path.join(dst_dir, "act_info.json"), "w") as f:
            json.dump(d, f)
        os.environ["BASS_ACT_ROOT_JSON_PATH"] = os.path.join(
            dst_dir, "act_info.json"
        )

        # Bass's insert_act_table_loads picks sets (and emits set IDs) from
        # its own read of act_info.json — keep it consistent with the
        # single-set file walrus will see.
        import concourse.bacc as bacc_mod
        import concourse.hw_specs as hw_specs

        full = hw_specs.get_activation_tables("gen3")
        filtered = {
            "natural_log_exp_and_others": full["natural_log_exp_and_others"]
        }

        def _tables_override(module_arch):
            return filtered

        hw_specs.get_activation_tables = _tables_override
        bacc_mod.get_activation_tables = _tables_override
    except Exception:
        pass  # fall back to the stock tables (3 extra table loads, ~4us)


def _build():
    from contextlib import ExitStack

    import concourse.bacc as bacc
    import concourse.tile as tile
    from concourse import mybir

    AF = mybir.ActivationFunctionType
    ALU = mybir.AluOpType
    f32 = mybir.dt.float32

    # Bacc (not raw Bass): its compile() runs generate_event_semaphores,
    # which legalizes multi-sem waits down to the TRN2 1-wait-per-inst limit.
    nc = bacc.Bacc("TRN2", target_bir_lowering=False, debug=False)
    x = nc.declare_dram_parameter("x", [R, D], f32, isOutput=False)
    y = nc.declare_dram_parameter("y", [R, D], f32, isOutput=False)
    out = nc.declare_dram_parameter("out", [1, 1], f32, isOutput=True)
    xt = x.rearrange("(t p) d -> t p d", p=P)
    yt = y.rearrange("(t p) d -> t p d", p=P)

    with ExitStack() as ctx:
        tc = ctx.enter_context(tile.TileContext(nc))
        xpool = ctx.enter_context(tc.tile_pool(name="xp", bufs=T))
        ypool = ctx.enter_context(tc.tile_pool(name="yp", bufs=T))
        scr = ctx.enter_context(tc.tile_pool(name="scr", bufs=1))
        stats = ctx.enter_context(tc.tile_pool(name="stats", bufs=1))
        psum = ctx.enter_context(tc.tile_pool(name="ps", bufs=1, space="PSUM"))

        # Elementwise-product dump targets; one per engine so the engines
        # never share a WAW dependency on them.
        scr_dve = scr.tile([P, D], f32, tag="scr_dve")
        scr_act = scr.tile([P, D], f32, tag="scr_act")

        # Per-row reduction columns, one column per row-tile. Separate
        # tiles per writing engine (xx: ACT, xy: DVE, yy split).
        xx = stats.tile([P, T], f32, tag="xx")
        xy = stats.tile([P, T], f32, tag="xy")
        yy_a = stats.tile([P, YA], f32, tag="yya")
        yy_d = stats.tile([P, T - YA], f32, tag="yyd")

        # Spread the 16 input loads across the three otherwise-idle
        # sequencers (SP, PE, POOL): SWDGE descriptor writing is ~1.3us per
        # [128,1024] dma_start, and a single issuing engine serializes it
        # (~9us before the rings even start draining). Issue in tile order
        # so tile 0 lands first.
        xtiles, ytiles = [], []
        for t in range(T):
            xtile = xpool.tile([P, D], f32)
            nc.sync.dma_start(out=xtile, in_=xt[t])
            ytile = ypool.tile([P, D], f32)
            nc.gpsimd.dma_start(out=ytile, in_=yt[t])
            xtiles.append(xtile)
            ytiles.append(ytile)

        for t in range(T):
            a, b = xtiles[t], ytiles[t]
            # scalar_tensor_tensor: out = (in0 op0 s) op1 in1, accum = sum(out)
            nc.vector.scalar_tensor_tensor(
                scr_dve, a, 1.0, b, op0=ALU.mult, op1=ALU.mult,
                accum_out=xy[:, t : t + 1],
            )
            nc.scalar.activation(scr_act, a, AF.Square, accum_out=xx[:, t : t + 1])
            if t < YA:
                nc.scalar.activation(
                    scr_act, b, AF.Square, accum_out=yy_a[:, t : t + 1]
                )
            else:
                nc.vector.scalar_tensor_tensor(
                    scr_dve, b, 1.0, b, op0=ALU.mult, op1=ALU.mult,
                    accum_out=yy_d[:, t - YA : t - YA + 1],
                )

        # cos = xy / sqrt(xx*yy); loss_row = ln(0.5*cos + 0.5); out = -mean.
        prod = stats.tile([P, T], f32, tag="prod")
        nc.vector.tensor_mul(prod[:, :YA], xx[:, :YA], yy_a)
        nc.vector.tensor_mul(prod[:, YA:], xx[:, YA:], yy_d)
        lp = stats.tile([P, T], f32, tag="lp")
        nc.scalar.activation(lp, prod, AF.Ln)
        rinv = stats.tile([P, T], f32, tag="rinv")
        nc.scalar.activation(rinv, lp, AF.Exp, scale=-0.5)  # prod**-0.5
        cos = stats.tile([P, T], f32, tag="cos")
        nc.vector.tensor_mul(cos, xy, rinv)
        half = stats.tile([P, 1], f32, tag="half")
        nc.vector.memset(half, 0.5)
        terms = stats.tile([P, T], f32, tag="terms")
        row_loss = stats.tile([P, 1], f32, tag="rl")
        nc.scalar.activation(
            terms, cos, AF.Ln, scale=0.5, bias=half, accum_out=row_loss
        )
        # Partition-axis sum via PE: [1,1] = row_loss.T @ (-1/N ones).
        wvec = stats.tile([P, 1], f32, tag="wv")
        nc.vector.memset(wvec, -1.0 / N)
        acc = psum.tile([1, 1], f32, tag="acc")
        nc.tensor.matmul(acc, row_loss, wvec, start=True, stop=True)
        res = stats.tile([1, 1], f32, tag="res")
        nc.scalar.copy(res, acc)
        nc.sync.dma_start(out=out[:, :], in_=res)

    nc.compile()
    return nc


def kernel(x, y, trace=False):
    from concourse.bass_utils import run_bass_kernel_spmd

    global _nc_cache, last_results
    _install_act_table_override()
    if _nc_cache is None:
        _nc_cache = _build()
    nc = _nc_cache

    x = np.ascontiguousarray(np.asarray(x), dtype=np.float32)
    y = np.ascontiguousarray(np.asarray(y), dtype=np.float32)
    in_maps = [
        {"x": x[c * R : (c + 1) * R], "y": y[c * R : (c + 1) * R]}
        for c in range(NCORES)
    ]
    results = run_bass_kernel_spmd(
        nc, in_maps, core_ids=list(range(NCORES)), trace=trace
    )
    last_results = results
    total = np.float32(0.0)
    for r in results.results:
        total = np.float32(total + r["out"][0, 0])
    return np.asarray(total, dtype=np.float32)
